# revision 2
# baseline (speedup 1.0000x reference)
"""NaryTreeLSTM Trainium2 kernel, v2.

Data-parallel over batch (32768 -> 4096 rows/core on 8 cores), transposed
[h, batch] on-device layout. Main changes vs v1:

- All x-side matmuls (child gates + node x-paths) run as fp8e4 DoubleRow
  (0.5 cyc/row) with host-side error compensation: z = x@W is computed as
  (2x)8@(16W)8 + (16E)8@(2W)8 + (x/2)8@(64R)8 where E = x - dec(x8),
  R = W - dec(W8); every product is 32*z so the ACT reads PSUM with
  scale=1/32. Scaling keeps all fp8 operands out of e4m3's denormal range
  (emulated end-to-end error: ~2.5e-3 vs fp32 reference).
- h-path (hsum@U) stays fp16 with weights pre-scaled by 32 into the same
  PSUM accumulation group.
- ch = co * tanh(cc) runs as ONE custom DVE op (deg-5 odd minimax poly on
  [-1,1]; cc = sigmoid*tanh is bounded), removing the tanh(cc) ACT pass
  and the separate multiply.
- Elementwise tails operate on ht-merged [128, 2, N] fp16 tiles (one DVE
  op instead of two; DVE 2x mode) and part of the c/h tail runs on the
  otherwise-idle Pool engine.
- Outputs are stored fp16 and upcast host-side.
"""

import sys

sys.path.insert(0, "/opt/trn_rl_repo")

import numpy as np
import ml_dtypes

B, K, I, H = 32768, 4, 256, 256
NCORES = 8
BLOC = B // NCORES
C = 512

# tanh deg-5 odd minimax coefficients on [-1, 1] (max err 3.9e-4)
TCA, TCB, TCC = 0.99716033, -0.3079716, 0.07279027

F8NP = ml_dtypes.float8_e4m3

_cache = {}


def _fp8(a):
    return a.astype(F8NP)


def _register_ch_op():
    """Register the CH_TANH_ANT custom DVE op: out = in1 * tanhpoly(in0)."""
    from concourse import dve_ops
    from concourse.dve_spec import Spec, Src0, Src1, C0, C1, C2, sq, lower
    from concourse.dve_uop import DveOpSpec

    name = "CH_TANH_ANT"
    if name in dve_ops._SUB_OPCODE_FOR_NAME:
        return next(o for o in dve_ops.OPS if o.name == name)
    u = sq(Src0)
    p = (u * C2 + C1) * u + C0
    body = (Src0 * p) * Src1

    def ref(in0, in1, s0, s1, imm2):
        t = in0.astype(np.float32)
        uu = t * t
        return (t * ((uu * imm2 + s1) * uu + s0) * in1).astype(np.float32)

    spec = Spec(body=body, reference=ref)
    row = max(dve_ops._SUB_OPCODE_FOR_NAME.values()) + 1
    assert row < 0x20
    dve_ops._SUB_OPCODE_FOR_NAME[name] = row
    shas = {}
    for ver in ("v3", "v4"):
        s = DveOpSpec(name=name, opcode=row, uops=lower(spec, ver=ver), rd1_en=True)
        shas[ver] = s.sha(ver)
    op = dve_ops.DveOp(name, spec, subdim=False, uops_sha=shas)
    dve_ops.OPS.append(op)
    dve_ops.CUSTOM_DVE_SPECS[name] = spec
    return op


def _build(nchunk):
    import concourse.bass as bass  # noqa: F401
    import concourse.tile as tile
    from concourse import bacc, mybir
    import contextlib

    f8 = mybir.dt.float8e4
    f16, f32 = mybir.dt.float16, mybir.dt.float32
    AF = mybir.ActivationFunctionType
    DR = mybir.MatmulPerfMode.DoubleRow
    ch_op = _register_ch_op()
    SC = 1.0 / 32.0

    nc = bacc.Bacc("TRN2", target_bir_lowering=False, debug=False, num_devices=NCORES)

    NCOL = nchunk * C
    # ax blocks (last dim): [X2: cx0..3,x | E16: cx0..3,x | Xh: cx0..3,x]
    ax = nc.dram_tensor("ax", [nchunk, 128, 2, 15 * C], f8, kind="ExternalInput").ap()
    # w8 cols: [16W | 2W | 64R] x [g*256 + ht*128 + m], g in (i,o,u,f)
    w8 = nc.dram_tensor("w8", [128, 2, 3072], f8, kind="ExternalInput").ap()
    # wh: 32*UhT, blocks [Ui|Uo|Uu|WfK0..3]; first dim = contraction tile
    wh = nc.dram_tensor("wh", [2, 128, 1792], f16, kind="ExternalInput").ap()
    bmat = nc.dram_tensor("bmat", [128, 8], f32, kind="ExternalInput").ap()
    h_out = nc.dram_tensor("h_out", [128, 2, NCOL], f16, kind="ExternalOutput").ap()
    c_out = nc.dram_tensor("c_out", [128, 2, NCOL], f16, kind="ExternalOutput").ap()

    with tile.TileContext(nc) as tc:
        with contextlib.ExitStack() as ctx:
            wpool = ctx.enter_context(tc.tile_pool(name="w", bufs=1))
            apool = ctx.enter_context(tc.tile_pool(name="a", bufs=3))
            gpool = ctx.enter_context(tc.tile_pool(name="g", bufs=1))
            opool = ctx.enter_context(tc.tile_pool(name="o", bufs=2))
            ppool = ctx.enter_context(tc.tile_pool(name="ps", bufs=2, space="PSUM"))

            w8_sb = wpool.tile([128, 2, 3072], f8, tag="w8", name="w8_sb")
            nc.gpsimd.dma_start(w8_sb[:], w8[:])
            whs = []
            for ct in range(2):
                t = wpool.tile([128, 1792], f16, tag=f"wh{ct}", name=f"wh{ct}")
                nc.gpsimd.dma_start(t[:], wh[ct])
                whs.append(t)
            b_sb = wpool.tile([128, 8], f32, tag="bias", name="b_sb")
            nc.gpsimd.dma_start(b_sb[:], bmat[:])
            # tiny dummy activation so the ACT table load happens during the
            # initial DMA wait
            warm = wpool.tile([1, 8], f32, tag="warm", name="warm")
            nc.vector.memset(warm[:], 0.0)
            nc.scalar.activation(warm[:], warm[:], AF.Sigmoid)

            def w16(g, ht):
                col = g * 256 + ht * 128
                return w8_sb[:, :, col : col + 128]

            def w2(g, ht):
                col = 1024 + g * 256 + ht * 128
                return w8_sb[:, :, col : col + 128]

            def r64(g, ht):
                col = 2048 + g * 256 + ht * 128
                return w8_sb[:, :, col : col + 128]

            def wu(ct, blk, ht):
                col = blk * 256 + ht * 128
                return whs[ct][:, col : col + 128]

            def bias(g, ht):
                return b_sb[:, 2 * g + ht : 2 * g + ht + 1]

            def psum4():
                return ppool.tile([128, 4 * C], f32, tag="p4", name="p4")

            # fp8 compensation variants: (lhsT accessor, ax column base)
            VARIANTS = ((w16, 0), (w2, 5 * C), (r64, 10 * C))

            def comp_mm(ps_out, g, ht, a_sb, lo, stop=True):
                """Accumulate 32*(x@Wg) for one C-wide x-block into one bank."""
                for v, (wv, base) in enumerate(VARIANTS):
                    nc.tensor.matmul(
                        ps_out, lhsT=wv(g, ht), rhs=a_sb[:, :, base + lo : base + lo + C],
                        start=(v == 0), stop=stop and (v == 2), perf_mode=DR,
                    )

            def emit_child(c):
                a_sb = apool.tile([128, 2, 15 * C], f8, tag="a", name="a_sb")
                nc.sync.dma_start(a_sb[:], ax[c])
                gates = []
                for g in range(3):  # i, o, u over child_x
                    gt = gpool.tile(
                        [128, 2, 4 * C], f16, tag=f"cg{g}", bufs=2, name=f"cg{g}"
                    )
                    fn = AF.Tanh if g == 2 else AF.Sigmoid
                    for ht in range(2):
                        ps = psum4()
                        for v, (wv, base) in enumerate(VARIANTS):
                            for k in range(4):
                                nc.tensor.matmul(
                                    ps[:, k * C : (k + 1) * C],
                                    lhsT=wv(g, ht),
                                    rhs=a_sb[:, :, base + k * C : base + (k + 1) * C],
                                    start=(v == 0), stop=(v == 2), perf_mode=DR,
                                )
                        nc.scalar.activation(
                            gt[:, ht, :], ps[:], fn, bias=bias(g, ht), scale=SC
                        )
                    gates.append(gt)
                return dict(c=c, a=a_sb, gates=gates)

            def emit_child_tail(st):
                ci, co, cu = st["gates"]
                cc = gpool.tile([128, 2, 4 * C], f16, tag="cc", bufs=2, name="cc")
                nc.vector.tensor_mul(cc[:], ci[:], cu[:])
                ch = gpool.tile([128, 2, 4 * C], f16, tag="ch", name="ch")
                flat = lambda ap: ap.rearrange("p a b -> p (a b)")
                nc.vector._custom_dve(
                    ch_op, out=flat(ch[:]), in0=flat(cc[:]), in1=flat(co[:]),
                    s0=TCA, s1=TCB, imm2=TCC,
                )
                t2 = gpool.tile([128, 2, 2 * C], f16, tag="t2", name="t2")
                hs = gpool.tile([128, 2, C], f16, tag="hs", bufs=3, name="hs")
                with nc.allow_low_precision("fp16 hsum tree"):
                    nc.vector.tensor_add(t2[:], ch[:, :, 0 : 2 * C], ch[:, :, 2 * C : 4 * C])
                    nc.vector.tensor_add(hs[:], t2[:, :, 0:C], t2[:, :, C : 2 * C])
                st["cc"] = cc
                st["hs"] = hs

            def emit_node(st):
                a_sb, hs = st["a"], st["hs"]
                xlo, xhi = 4 * C, 5 * C
                i_sb = gpool.tile([128, 2, C], f16, tag="i", name="i_sb")
                o_sb = gpool.tile([128, 2, C], f16, tag="o", bufs=2, name="o_sb")
                u_sb = gpool.tile([128, 2, C], f16, tag="u", name="u_sb")
                f_sb = gpool.tile([128, 2, 4 * C], f16, tag="f", name="f_sb")
                for ht in range(2):
                    ps = psum4()  # banks: i | o | u | (spare)
                    for g in range(3):
                        out = ps[:, g * C : (g + 1) * C]
                        comp_mm(out, g, ht, a_sb, xlo, stop=False)
                        nc.tensor.matmul(
                            out, lhsT=wu(0, g, ht), rhs=hs[:, 0, :],
                            start=False, stop=False,
                        )
                        nc.tensor.matmul(
                            out, lhsT=wu(1, g, ht), rhs=hs[:, 1, :],
                            start=False, stop=True,
                        )
                    nc.scalar.activation(
                        i_sb[:, ht, :], ps[:, 0:C], AF.Sigmoid, bias=bias(0, ht),
                        scale=SC,
                    )
                    nc.scalar.activation(
                        o_sb[:, ht, :], ps[:, C : 2 * C], AF.Sigmoid, bias=bias(1, ht),
                        scale=SC,
                    )
                    nc.scalar.activation(
                        u_sb[:, ht, :], ps[:, 2 * C : 3 * C], AF.Tanh, bias=bias(2, ht),
                        scale=SC,
                    )
                    psf = psum4()
                    for k in range(4):
                        out = psf[:, k * C : (k + 1) * C]
                        comp_mm(out, 3, ht, a_sb, xlo, stop=False)
                        nc.tensor.matmul(
                            out, lhsT=wu(0, 3 + k, ht), rhs=hs[:, 0, :],
                            start=False, stop=False,
                        )
                        nc.tensor.matmul(
                            out, lhsT=wu(1, 3 + k, ht), rhs=hs[:, 1, :],
                            start=False, stop=True,
                        )
                    nc.scalar.activation(
                        f_sb[:, ht, :], psf[:], AF.Sigmoid, bias=bias(3, ht), scale=SC
                    )
                st["iouf"] = (i_sb, o_sb, u_sb, f_sb)

            def emit_node_tail(st):
                c = st["c"]
                i_sb, o_sb, u_sb, f_sb = st["iouf"]
                cc = st["cc"]
                iu = gpool.tile([128, 2, C], f16, tag="iu", name="iu")
                nc.vector.tensor_mul(iu[:], i_sb[:], u_sb[:])
                fcc = gpool.tile([128, 2, 4 * C], f16, tag="fcc", name="fcc")
                nc.vector.tensor_mul(fcc[:], f_sb[:], cc[:])
                t1 = gpool.tile([128, 2, 2 * C], f16, tag="t1", name="t1")
                t2n = gpool.tile([128, 2, C], f16, tag="t2n", name="t2n")
                c_sb = opool.tile([128, 2, C], f16, tag="c", name="c_sb")
                with nc.allow_low_precision("fp16 c tree"):
                    nc.vector.tensor_add(
                        t1[:], fcc[:, :, 0 : 2 * C], fcc[:, :, 2 * C : 4 * C]
                    )
                    nc.gpsimd.tensor_add(t2n[:], t1[:, :, 0:C], t1[:, :, C : 2 * C])
                    nc.gpsimd.tensor_add(c_sb[:], t2n[:], iu[:])
                tc_sb = gpool.tile([128, 2, C], f16, tag="tc", name="tc_sb")
                nc.scalar.activation(tc_sb[:], c_sb[:], AF.Tanh)
                h_sb = opool.tile([128, 2, C], f16, tag="h", name="h_sb")
                nc.gpsimd.tensor_mul(h_sb[:], o_sb[:], tc_sb[:])
                cols = slice(c * C, (c + 1) * C)
                nc.sync.dma_start(h_out[:, :, cols], h_sb[:])
                nc.sync.dma_start(c_out[:, :, cols], c_sb[:])

            prev = None
            for c in range(nchunk):
                cur = emit_child(c)
                if prev is not None:
                    emit_node(prev)
                emit_child_tail(cur)
                if prev is not None:
                    emit_node_tail(prev)
                prev = cur
            emit_node(prev)
            emit_node_tail(prev)

    nc.compile()
    return nc


def _prep_shared(Wi, bi, Wf, bf, Wo, bo, Wu, bu, Ui, Uo, Uu, WfK):
    Wx = np.stack([Wi, Wo, Wu, Wf]).astype(np.float32)  # [4, H, I], g=(i,o,u,f)
    W16 = _fp8(16.0 * Wx)
    R = Wx - W16.astype(np.float32) / 16.0
    W2 = _fp8(2.0 * Wx)
    R64 = _fp8(64.0 * R)

    def pack(V):  # [4, 256, 256] -> [128, 2, 1024]: [p, it, g*256+ht*128+m]
        v = V.reshape(4, 2, 128, 2, 128)  # [g, ht, m, it, p]
        return np.ascontiguousarray(v.transpose(4, 3, 0, 1, 2).reshape(128, 2, 1024))

    w8 = np.concatenate([pack(W16), pack(W2), pack(R64)], axis=2)

    Uall = np.concatenate(
        [Ui, Uo, Uu, WfK[0], WfK[1], WfK[2], WfK[3]], axis=0
    ).astype(np.float32)  # [1792, 256]
    wh = np.ascontiguousarray(
        (32.0 * Uall.T).astype(np.float16).reshape(2, 128, 1792)
    )

    bmat = np.empty((128, 8), np.float32)
    for g, b in enumerate([bi, bo, bu, bf]):
        b = np.asarray(b, np.float32)
        bmat[:, 2 * g] = b[:128]
        bmat[:, 2 * g + 1] = b[128:]
    return w8, wh, bmat


def _prep_core(x, child_x, m, nchunk):
    bloc = nchunk * C
    sl = slice(m * bloc, (m + 1) * bloc)
    cxt = np.asarray(child_x[sl], np.float32).transpose(2, 1, 0)  # [256, 4, bloc]
    xt = np.asarray(x[sl], np.float32).T[:, None, :]  # [256, 1, bloc]
    full = np.concatenate([cxt, xt], axis=1)  # [256, 5, bloc]
    X2 = _fp8(2.0 * full)
    E = full - X2.astype(np.float32) / 2.0
    E16 = _fp8(16.0 * E)
    Xh = _fp8(0.5 * full)

    def pk(V):  # [256, 5, bloc] -> [nchunk, 128, 2, 5C]
        v = V.reshape(2, 128, 5, nchunk, C).transpose(3, 1, 0, 2, 4)
        return v.reshape(nchunk, 128, 2, 5 * C)

    return np.ascontiguousarray(np.concatenate([pk(X2), pk(E16), pk(Xh)], axis=3))


def _run(inputs, nchunk, trace=False):
    from concourse.bass_utils import run_bass_kernel_spmd

    key = ("nc", nchunk)
    if key not in _cache:
        _cache[key] = _build(nchunk)
    nc = _cache[key]

    w8, wh, bmat = _prep_shared(
        inputs["Wi"], inputs["bi"], inputs["Wf"], inputs["bf"],
        inputs["Wo"], inputs["bo"], inputs["Wu"], inputs["bu"],
        inputs["Ui"], inputs["Uo"], inputs["Uu"], inputs["WfK"],
    )
    in_maps = []
    for m in range(NCORES):
        axm = _prep_core(inputs["x"], inputs["child_x"], m, nchunk)
        in_maps.append({"ax": axm, "w8": w8, "wh": wh, "bmat": bmat})

    res = run_bass_kernel_spmd(
        nc, in_maps, core_ids=list(range(NCORES)), trace=trace
    )
    bloc = nchunk * C
    h = np.empty((NCORES * bloc, 256), np.float32)
    c = np.empty((NCORES * bloc, 256), np.float32)
    for m, r in enumerate(res.results):
        # [128, 2, bloc] fp16 -> [bloc, 256]
        h[m * bloc : (m + 1) * bloc] = (
            np.asarray(r["h_out"]).transpose(2, 1, 0).reshape(bloc, 256).astype(np.float32)
        )
        c[m * bloc : (m + 1) * bloc] = (
            np.asarray(r["c_out"]).transpose(2, 1, 0).reshape(bloc, 256).astype(np.float32)
        )
    return (h, c), res


def kernel(**inputs):
    (h, c), _ = _run(inputs, BLOC // C)
    return h, c


# revision 4
# speedup vs baseline: 1.2141x; 1.2141x over previous
"""NaryTreeLSTM Trainium2 kernel.

Strategy: pure data-parallel over batch (B=32768 -> 4096 rows/core on 8
cores). All on-device compute happens in transposed [h, batch] layout so
matmuls contract over the SBUF partition dim; activations are pre-cast to
fp16 host-side (halves DMA, 4x faster PE than fp32). Gate preactivations
accumulate in PSUM (x-path + hsum-path summed for free; biases fused into
the ACT instruction), nonlinearities run on the scalar engine straight out
of 4-bank [128,2048] PSUM tiles (child gates merged over all 4 children,
same per-partition bias), and the k-reductions (hsum, c = i*u +
sum_k f_k*cc_k) are short DVE tree-adds. Emission is software-pipelined
per 512-column chunk as child_gates(c) | node_gates(c-1) | child_tail(c)
| node_tail(c-1) so the scalar engine (the bottleneck at ~196us/core
busy; PE ~194us) never waits on a DVE chain. Measured: ~247us HW exec,
rel err ~1e-3 vs the fp32 reference.
"""

import sys

sys.path.insert(0, "/opt/trn_rl_repo")

import numpy as np

B, K, I, H = 32768, 4, 256, 256
NCORES = 8
BLOC = B // NCORES  # 4096 batch rows per core
C = 512  # chunk columns (one PSUM bank of fp32)

_cache = {}


def _build(nchunk):
    """Build the per-core Bass program (identical on all cores)."""
    import concourse.bass as bass  # noqa: F401
    import bass_rust as _bass_rust
    import concourse.tile as tile
    from concourse import bacc, mybir

    f16, f32 = mybir.dt.float16, mybir.dt.float32
    AF = mybir.ActivationFunctionType

    nc = bacc.Bacc("TRN2", target_bir_lowering=False, debug=False, num_devices=NCORES)

    # DRAM I/O. ax packs, per (chunk, itile): [cx_k0|cx_k1|cx_k2|cx_k3|x]
    # blocks of C columns each, rows = 128 contraction indices.
    ax = nc.dram_tensor("ax", [nchunk, 2, 128, 5 * C], f16, kind="ExternalInput").ap()
    # wcat cols: 0:768 WxiouT (g*256+h), 768:1024 WfT, 1024:2816 UhT
    # ([Ui|Uo|Uu|WfK0..3] at 1024+blk*256+h); rows = contraction index.
    wcat = nc.dram_tensor("wcat", [2, 128, 2816], f16, kind="ExternalInput").ap()
    # bmat cols: 2g+t for g in {i,o,u,f}, t = h-tile
    bmat = nc.dram_tensor("bmat", [128, 8], f32, kind="ExternalInput").ap()
    h_out = nc.dram_tensor("h_out", [256, nchunk * C], f32, kind="ExternalOutput").ap()
    c_out = nc.dram_tensor("c_out", [256, nchunk * C], f32, kind="ExternalOutput").ap()

    with tile.TileContext(nc) as tc:
        import contextlib

        with contextlib.ExitStack() as ctx:
            wpool = ctx.enter_context(tc.tile_pool(name="w", bufs=1))
            apool = ctx.enter_context(tc.tile_pool(name="a", bufs=4))
            gpool = ctx.enter_context(tc.tile_pool(name="g", bufs=1))
            opool = ctx.enter_context(tc.tile_pool(name="o", bufs=2))
            ppool = ctx.enter_context(tc.tile_pool(name="ps", bufs=2, space="PSUM"))

            # weights on the gpsimd DMA queue so the first ax load (sync
            # queue) runs concurrently; x-path weights first so child
            # matmuls can start before the U-path weights arrive.
            wA, wB = [], []
            for it in range(2):
                a_ = wpool.tile([128, 1024], f16, tag=f"wA{it}", name=f"wA{it}")
                nc.gpsimd.dma_start(a_[:], wcat[it, :, 0:1024])
                wA.append(a_)
            for it in range(2):
                b_ = wpool.tile([128, 1792], f16, tag=f"wB{it}", name=f"wB{it}")
                nc.gpsimd.dma_start(b_[:], wcat[it, :, 1024:2816])
                wB.append(b_)
            b_sb = wpool.tile([128, 8], f32, tag="bias", name="b_sb")
            nc.gpsimd.dma_start(b_sb[:], bmat[:])
            # tiny dummy activations so the ACT table load (~1.3us) happens
            # during the initial DMA wait instead of before the first gate
            warm = wpool.tile([1, 8], f32, tag="warm", name="warm")
            nc.vector.memset(warm[:], 0.0)
            nc.scalar.activation(warm[:], warm[:], AF.Sigmoid)

            def wx(it, col):
                return wA[it][:, col : col + 128]

            def wu(it, col):
                return wB[it][:, col - 1024 : col - 1024 + 128]

            def bias(g, ht):
                # g: 0=i, 1=o, 2=u, 3=f
                return b_sb[:, 2 * g + ht : 2 * g + ht + 1]

            def psum4():
                # uniform 4-bank PSUM tiles; bufs=2 -> all 8 banks in flight
                return ppool.tile([128, 4 * C], f32, tag="p4", name="p4")

            def emit_child(c):
                """Child (leaf) phase for chunk c. Returns live tiles."""
                a_sb = []
                for it in range(2):
                    a = apool.tile([128, 5 * C], f16, tag=f"a{it}", name=f"a{it}")
                    nc.sync.dma_start(a[:], ax[c, it])
                    a_sb.append(a)
                gates = {}  # (g, ht) -> [128, 4C] fp16 (4 k-blocks)
                for ht in range(2):
                    for g in range(3):
                        gt = gpool.tile(
                            [128, 4 * C], f16, tag=f"cg{g}{ht}", bufs=2,
                            name=f"cg{g}{ht}",
                        )
                        gates[(g, ht)] = gt
                        col = g * 256 + ht * 128
                        fn = AF.Tanh if g == 2 else AF.Sigmoid
                        ps = psum4()
                        for it in range(2):  # it-major: same lhsT for 4 MMs
                            for k in range(4):
                                nc.tensor.matmul(
                                    ps[:, k * C : (k + 1) * C],
                                    lhsT=wx(it, col),
                                    rhs=a_sb[it][:, k * C : (k + 1) * C],
                                    start=(it == 0),
                                    stop=(it == 1),
                                )
                        nc.scalar.activation(gt[:], ps[:], fn, bias=bias(g, ht))
                return dict(c=c, a=a_sb, gates=gates)

            def emit_child_tail(st):
                gates = st["gates"]
                ucc, hs = {}, {}
                for ht in range(2):
                    # ucc = [u_node | cc0..cc3]; cc written now, u in node phase
                    ucc[ht] = gpool.tile(
                        [128, 5 * C], f16, tag=f"ucc{ht}", bufs=3, name=f"ucc{ht}"
                    )
                    nc.vector.tensor_mul(
                        ucc[ht][:, C : 5 * C], gates[(0, ht)][:], gates[(2, ht)][:]
                    )
                    tcc = gpool.tile([128, 4 * C], f16, tag=f"tcc{ht}", name="tcc")
                    nc.scalar.activation(tcc[:], ucc[ht][:, C : 5 * C], AF.Tanh)
                    prod = gpool.tile([128, 4 * C], f16, tag=f"prod{ht}", name="prod")
                    nc.vector.tensor_mul(prod[:], gates[(1, ht)][:], tcc[:])
                    # hsum = sum_k prod_k, via tree adds
                    t2 = gpool.tile([128, 2 * C], f16, tag=f"t2{ht}", name="t2")
                    nc.vector.tensor_add(t2[:], prod[:, 0 : 2 * C], prod[:, 2 * C : 4 * C])
                    hs[ht] = gpool.tile(
                        [128, C], f16, tag=f"hs{ht}", bufs=3, name=f"hs{ht}"
                    )
                    with nc.allow_low_precision("hsum kept in fp16 for matmul rhs"):
                        nc.vector.tensor_add(hs[ht][:], t2[:, 0:C], t2[:, C : 2 * C])
                st["ucc"] = ucc
                st["hs"] = hs

            def emit_node(st):
                c, a_sb, ucc, hs = st["c"], st["a"], st["ucc"], st["hs"]
                xsl = slice(4 * C, 5 * C)

                for ht in range(2):
                    ifff = gpool.tile([128, 5 * C], f16, tag=f"ifff{ht}", name="ifff")
                    o_sb = gpool.tile([128, C], f16, tag=f"o{ht}", name="o_sb")
                    # i, o, u gates share one 4-bank tile [i|o|u|unused]
                    ps_iou = psum4()
                    for it in range(2):
                        for g in range(3):
                            nc.tensor.matmul(
                                ps_iou[:, g * C : (g + 1) * C],
                                lhsT=wx(it, g * 256 + ht * 128),
                                rhs=a_sb[it][:, xsl],
                                start=(it == 0),
                                stop=False,
                            )
                    for ct in range(2):
                        for g in range(3):
                            nc.tensor.matmul(
                                ps_iou[:, g * C : (g + 1) * C],
                                lhsT=wu(ct, 1024 + g * 256 + ht * 128),
                                rhs=hs[ct][:],
                                start=False,
                                stop=(ct == 1),
                            )
                    nc.scalar.activation(
                        ifff[:, 0:C], ps_iou[:, 0:C], AF.Sigmoid, bias=bias(0, ht)
                    )
                    nc.scalar.activation(
                        o_sb[:], ps_iou[:, C : 2 * C], AF.Sigmoid, bias=bias(1, ht)
                    )
                    nc.scalar.activation(
                        ucc[ht][:, 0:C], ps_iou[:, 2 * C : 3 * C], AF.Tanh,
                        bias=bias(2, ht),
                    )
                    # f gates for all 4 children in one 4-bank tile (the
                    # fx = Wf@x term is re-accumulated per k on the PE; the
                    # redundant matmuls are cheaper than any add elsewhere)
                    psf = psum4()
                    for it in range(2):
                        for k in range(4):
                            nc.tensor.matmul(
                                psf[:, k * C : (k + 1) * C],
                                lhsT=wx(it, 768 + ht * 128),
                                rhs=a_sb[it][:, xsl],
                                start=(it == 0),
                                stop=False,
                            )
                    for ct in range(2):
                        for k in range(4):
                            nc.tensor.matmul(
                                psf[:, k * C : (k + 1) * C],
                                lhsT=wu(ct, 1792 + k * 256 + ht * 128),
                                rhs=hs[ct][:],
                                start=False,
                                stop=(ct == 1),
                            )
                    nc.scalar.activation(
                        ifff[:, C : 5 * C], psf[:], AF.Sigmoid, bias=bias(3, ht)
                    )
                    st.setdefault("ifff", {})[ht] = ifff
                    st.setdefault("o_sb", {})[ht] = o_sb

            def emit_node_tail(st):
                c, ucc = st["c"], st["ucc"]
                for ht in range(2):
                    ifff, o_sb = st["ifff"][ht], st["o_sb"][ht]
                    # c = i*u + sum_k f_k*cc_k via one mult + tree adds (f32)
                    prod5 = gpool.tile([128, 5 * C], f16, tag=f"p5{ht}", name="prod5")
                    nc.vector.tensor_mul(prod5[:], ifff[:], ucc[ht][:])
                    t1 = gpool.tile([128, 2 * C], f32, tag=f"t1{ht}", name="t1")
                    nc.vector.tensor_add(
                        t1[:], prod5[:, C : 3 * C], prod5[:, 3 * C : 5 * C]
                    )
                    t2 = gpool.tile([128, C], f32, tag=f"t2n{ht}", name="t2n")
                    nc.vector.tensor_add(t2[:], t1[:, 0:C], t1[:, C : 2 * C])
                    c_sb = opool.tile([128, C], f32, tag=f"c{ht}", name="c_sb")
                    nc.vector.tensor_add(c_sb[:], t2[:], prod5[:, 0:C])
                    tc_sb = gpool.tile([128, C], f16, tag=f"tc{ht}", name="tc_sb")
                    nc.scalar.activation(tc_sb[:], c_sb[:], AF.Tanh)
                    h_sb = opool.tile([128, C], f32, tag=f"h{ht}", name="h_sb")
                    nc.vector.tensor_mul(h_sb[:], o_sb[:], tc_sb[:])
                    rows = slice(ht * 128, (ht + 1) * 128)
                    cols = slice(c * C, (c + 1) * C)
                    nc.sync.dma_start(h_out[rows, cols], h_sb[:])
                    nc.sync.dma_start(c_out[rows, cols], c_sb[:])

            # Software pipeline per step c:
            #   child_gates(c) | node_gates(c-1) | child_tail(c) | node_tail(c-1)
            # ACT order interleaves so every DVE chain (cc-mult, c-tree)
            # runs under unrelated ACT work.
            prev = None
            for c in range(nchunk):
                cur = emit_child(c)
                if prev is not None:
                    emit_node(prev)
                emit_child_tail(cur)
                if prev is not None:
                    emit_node_tail(prev)
                prev = cur
            emit_node(prev)
            emit_node_tail(prev)

    nc.compile()
    return nc


def _prep_shared(Wi, bi, Wf, bf, Wo, bo, Wu, bu, Ui, Uo, Uu, WfK):
    """Weight/bias packing shared by all cores."""
    WxiouT = np.concatenate([Wi, Wo, Wu], axis=0).T  # [256, 768]
    WfT = np.asarray(Wf).T  # [256, 256]
    UhT = np.concatenate([Ui, Uo, Uu, WfK[0], WfK[1], WfK[2], WfK[3]], axis=0).T
    wcat = np.concatenate([WxiouT, WfT, UhT], axis=1).astype(np.float16)  # [256, 2816]
    wcat = np.ascontiguousarray(wcat.reshape(2, 128, 2816))

    bmat = np.empty((128, 8), np.float32)
    for g, b in enumerate([bi, bo, bu, bf]):
        b = np.asarray(b, np.float32)
        bmat[:, 2 * g] = b[:128]
        bmat[:, 2 * g + 1] = b[128:]
    return wcat, bmat


def _prep_core(x, child_x, m, nchunk):
    """Pack per-core activations: [nchunk, 2, 128, 5C] fp16."""
    bloc = nchunk * C
    sl = slice(m * bloc, (m + 1) * bloc)
    cxt = np.asarray(child_x[sl], np.float16).transpose(2, 1, 0)  # [256, 4, bloc]
    xt = np.asarray(x[sl], np.float16).T[:, None, :]  # [256, 1, bloc]
    full = np.concatenate([cxt, xt], axis=1)  # [256, 5, bloc]
    # [it, p, j, chunk, cb] -> [chunk, it, p, j, cb]
    full = full.reshape(2, 128, 5, nchunk, C).transpose(3, 0, 1, 2, 4)
    return np.ascontiguousarray(full).reshape(nchunk, 2, 128, 5 * C)


def _run(inputs, nchunk, trace=False):
    from concourse.bass_utils import run_bass_kernel_spmd

    key = ("nc", nchunk)
    if key not in _cache:
        _cache[key] = _build(nchunk)
    nc = _cache[key]

    wcat, bmat = _prep_shared(
        inputs["Wi"], inputs["bi"], inputs["Wf"], inputs["bf"],
        inputs["Wo"], inputs["bo"], inputs["Wu"], inputs["bu"],
        inputs["Ui"], inputs["Uo"], inputs["Uu"], inputs["WfK"],
    )
    in_maps = []
    for m in range(NCORES):
        ax = _prep_core(inputs["x"], inputs["child_x"], m, nchunk)
        in_maps.append({"ax": ax, "wcat": wcat, "bmat": bmat})

    res = run_bass_kernel_spmd(
        nc, in_maps, core_ids=list(range(NCORES)), trace=trace
    )
    bloc = nchunk * C
    h = np.empty((NCORES * bloc, 256), np.float32)
    c = np.empty((NCORES * bloc, 256), np.float32)
    for m, r in enumerate(res.results):
        h[m * bloc : (m + 1) * bloc] = r["h_out"].T
        c[m * bloc : (m + 1) * bloc] = r["c_out"].T
    return (h, c), res


def kernel(**inputs):
    (h, c), _ = _run(inputs, BLOC // C)
    return h, c

